# revision 3
# baseline (speedup 1.0000x reference)
"""Trainium2 Bass kernel for nn_AttentionSeqToMasked (dense transformer attention).

Full-input contract: kernel(**inputs) takes the unsharded numpy inputs and
returns the full [B, SQ, H*D_V] float32 output.

Sharding (8 cores): data parallel over batch (B=4 -> 2 cores per batch) x
tensor parallel over heads (16 heads -> 8 per core). Each core computes
attention for one (batch, head-half) pair; host gathers the slices.

Per-core dataflow (all matmuls bf16 inputs, fp32 PSUM accumulation):
  - Host pre-transposes activations to X^T [D_PRE, S] bf16 so the contraction
    dim (D_PRE) lands on SBUF partitions with fully-contiguous DMA loads.
  - Projections compute q^T/k^T = W^T @ X^T directly (head-dim on partitions),
    v in natural [s, d_v] layout with a ones-column appended via the weight
    matrix (W_ext column of zeros + bias 1.0).
  - Scores are computed transposed: scoresT[k, q] = kT.T @ qT, two heads
    packed into the 128x128 PE array per matmul pair (d_head=64 row groups).
  - Key-mask folds into the exp as a per-partition bias (0 or -30000);
    1/sqrt(d) folds into the exp scale. No max-subtraction is needed
    (logits are O(1) by construction; exp cannot overflow fp32).
  - AV matmul contracts exp(scores)T with [v | ones]: row 64 of the psum is
    the softmax denominator, computed for free alongside the numerator.
  - A final PE transpose returns [q, d_v+1] tiles; VectorE multiplies by the
    reciprocal denominator and the result DMAs straight to DRAM.
"""

import os

import numpy as np
import ml_dtypes

import concourse.bass as bass
import concourse.bacc as bacc
import concourse.mybir as mybir
import concourse.tile as tile
from concourse.bass_utils import run_bass_kernel_spmd
from concourse.masks import make_identity

# Problem shape (hardcoded per contract)
B, SQ, SK = 4, 2048, 2048
D_PRE = 1024
H, D_QK, D_V = 16, 64, 64
N_CORES = 8
HALF = (H // 2) * D_QK  # 512 columns of the projection handled per core
N_PAIRS = 4  # head pairs per core
S_CHUNK = 512  # moving free-dim per matmul
N_DT = D_PRE // 128  # d_pre tiles of 128
N_KT = SK // 128  # key tiles of 128
N_QC = SQ // S_CHUNK  # query chunks of 512
MASK_NEG = -30000.0

F32 = mybir.dt.float32
BF16 = mybir.dt.bfloat16
BF16_NP = np.dtype(ml_dtypes.bfloat16)

_COMPILED = None


def _build_program():
    nc = bacc.Bacc("TRN2", target_bir_lowering=False, debug=False)

    # DRAM I/O (names are the in_map keys)
    xq = nc.dram_tensor("xq", [D_PRE, SQ], BF16, kind="ExternalInput").ap()
    xk = nc.dram_tensor("xk", [D_PRE, SK], BF16, kind="ExternalInput").ap()
    xv = nc.dram_tensor("xv", [D_PRE, SK], BF16, kind="ExternalInput").ap()
    wq = nc.dram_tensor("wq", [D_PRE, HALF], BF16, kind="ExternalInput").ap()
    wk = nc.dram_tensor("wk", [D_PRE, HALF], BF16, kind="ExternalInput").ap()
    # v weights with a zero column appended per head (ones column generator)
    wv = nc.dram_tensor("wv", [D_PRE, N_PAIRS * 130], BF16, kind="ExternalInput").ap()
    bq = nc.dram_tensor("bq", [128, N_PAIRS], F32, kind="ExternalInput").ap()
    bk = nc.dram_tensor("bk", [128, N_PAIRS], F32, kind="ExternalInput").ap()
    bv = nc.dram_tensor("bv", [128, N_PAIRS * 130], F32, kind="ExternalInput").ap()
    mb = nc.dram_tensor("mb", [128, N_KT], F32, kind="ExternalInput").ap()
    out = nc.dram_tensor("out", [SQ, HALF], F32, kind="ExternalOutput").ap()

    with tile.TileContext(nc) as tc:
        _emit(tc, xq, xk, xv, wq, wk, wv, bq, bk, bv, mb, out)

    nc.compile()
    return nc


def _emit(tc, xq, xk, xv, wq, wk, wv, bq, bk, bv, mb, out):
    nc = tc.nc
    from contextlib import ExitStack

    with ExitStack() as ctx:
        # ---- pools ----
        xp = ctx.enter_context(tc.tile_pool(name="x", bufs=3 * N_DT))
        wp = ctx.enter_context(tc.tile_pool(name="w", bufs=1))
        cp = ctx.enter_context(tc.tile_pool(name="const", bufs=1))
        qkvp = ctx.enter_context(tc.tile_pool(name="qkv", bufs=1))
        expp = ctx.enter_context(tc.tile_pool(name="exp", bufs=3))
        avtp = ctx.enter_context(tc.tile_pool(name="avt", bufs=2))
        stgp = ctx.enter_context(tc.tile_pool(name="stg", bufs=4))
        rp = ctx.enter_context(tc.tile_pool(name="recip", bufs=4))

        proj_ps = ctx.enter_context(tc.tile_pool(name="proj_ps", bufs=1, space="PSUM"))
        sc_ps = ctx.enter_context(tc.tile_pool(name="sc_ps", bufs=2, space="PSUM"))
        av_ps = ctx.enter_context(tc.tile_pool(name="av_ps", bufs=2, space="PSUM"))
        tp_ps = ctx.enter_context(tc.tile_pool(name="tp_ps", bufs=1, space="PSUM"))

        # ---- constants ----
        ident = cp.tile([128, 128], F32, name="ident")
        make_identity(nc, ident)
        mb_sb = cp.tile([128, N_KT], F32, name="mb_sb")
        nc.sync.dma_start(mb_sb, mb)
        bq_sb = cp.tile([128, N_PAIRS], F32, name="bq_sb")
        nc.sync.dma_start(bq_sb, bq)
        bk_sb = cp.tile([128, N_PAIRS], F32, name="bk_sb")
        nc.sync.dma_start(bk_sb, bk)
        bv_sb = cp.tile([128, N_PAIRS * 130], F32, name="bv_sb")
        nc.sync.dma_start(bv_sb, bv)

        # ---- weights ----
        wq_sb = []
        wk_sb = []
        wv_sb = []
        for dt_i in range(N_DT):
            t = wp.tile([128, HALF], BF16, name=f"wq{dt_i}", tag=f"wq{dt_i}")
            nc.sync.dma_start(t, wq[dt_i * 128 : (dt_i + 1) * 128, :])
            wq_sb.append(t)
            t = wp.tile([128, HALF], BF16, name=f"wk{dt_i}", tag=f"wk{dt_i}")
            nc.sync.dma_start(t, wk[dt_i * 128 : (dt_i + 1) * 128, :])
            wk_sb.append(t)
            t = wp.tile([128, N_PAIRS * 130], BF16, name=f"wv{dt_i}", tag=f"wv{dt_i}")
            nc.sync.dma_start(t, wv[dt_i * 128 : (dt_i + 1) * 128, :])
            wv_sb.append(t)

        # ---- activations X^T (bf16, contraction dim on partitions) ----
        def load_x(xap, pfx):
            ts = []
            for dt_i in range(N_DT):
                t = xp.tile([128, SQ], BF16, name=f"{pfx}{dt_i}", tag="x")
                nc.sync.dma_start(t, xap[dt_i * 128 : (dt_i + 1) * 128, :])
                ts.append(t)
            return ts

        xq_sb = load_x(xq, "xq")
        xk_sb = load_x(xk, "xk")
        xv_sb = load_x(xv, "xv")

        v_tiles = {}  # (pair, kt) -> [128, 130] bf16 tile

        def emit_v_group(g):
            # v projection for pairs 2g, 2g+1: out[s, 260] = Xv^T.T @ Wv_ext
            for st in range(N_KT):
                ps = proj_ps.tile([128, 512], F32, name=f"vps{g}_{st}", tag="proj")
                for dt_i in range(N_DT):
                    nc.tensor.matmul(
                        ps[:, 0:260],
                        lhsT=xv_sb[dt_i][:, st * 128 : (st + 1) * 128],
                        rhs=wv_sb[dt_i][:, g * 260 : (g + 1) * 260],
                        start=(dt_i == 0),
                        stop=(dt_i == N_DT - 1),
                    )
                for j in range(2):
                    pair = 2 * g + j
                    vt = qkvp.tile([128, 130], BF16, name=f"v{pair}_{st}", tag="v", bufs=2 * N_KT)
                    nc.vector.tensor_add(
                        vt,
                        ps[:, j * 130 : (j + 1) * 130],
                        bv_sb[:, pair * 130 : (pair + 1) * 130],
                    )
                    v_tiles[(pair, st)] = vt

        def emit_qk_proj(pair, w_sb, b_sb, pfx):
            # out^T[hd128, s] accumulated over d tiles; bias added on the copy
            dst = qkvp.tile([128, SQ], BF16, name=f"{pfx}T{pair}", tag=f"{pfx}T", bufs=2)
            for qc in range(N_QC):
                ps = proj_ps.tile([128, S_CHUNK], F32, name=f"{pfx}ps{pair}_{qc}", tag="proj")
                for dt_i in range(N_DT):
                    nc.tensor.matmul(
                        ps,
                        lhsT=w_sb[dt_i][:, pair * 128 : (pair + 1) * 128],
                        rhs=xq_sb[dt_i][:, qc * S_CHUNK : (qc + 1) * S_CHUNK]
                        if pfx == "q"
                        else xk_sb[dt_i][:, qc * S_CHUNK : (qc + 1) * S_CHUNK],
                        start=(dt_i == 0),
                        stop=(dt_i == N_DT - 1),
                    )
                nc.vector.tensor_scalar_add(
                    dst[:, qc * S_CHUNK : (qc + 1) * S_CHUNK],
                    ps,
                    b_sb[:, pair : pair + 1],
                )
            return dst

        for pair in range(N_PAIRS):
            if pair % 2 == 0:
                emit_v_group(pair // 2)
            qT = emit_qk_proj(pair, wq_sb, bq_sb, "q")
            kT = emit_qk_proj(pair, wk_sb, bk_sb, "k")

            # ---- attention for this pair ----
            for qc in range(N_QC):
                av_a = av_ps.tile([65, S_CHUNK], F32, name=f"ava{pair}_{qc}", tag="av")
                av_b = av_ps.tile([65, S_CHUNK], F32, name=f"avb{pair}_{qc}", tag="av")
                for kt in range(N_KT):
                    sc = sc_ps.tile([128, 1024], F32, name=f"sc{pair}_{qc}_{kt}", tag="sc")
                    # scoresT for heads A and B, packed in PE row groups
                    nc.tensor.matmul(
                        sc[:, 0:512],
                        lhsT=kT[0:64, kt * 128 : (kt + 1) * 128],
                        rhs=qT[0:64, qc * S_CHUNK : (qc + 1) * S_CHUNK],
                        start=True,
                        stop=True,
                    )
                    nc.tensor.matmul(
                        sc[:, 512:1024],
                        lhsT=kT[64:128, kt * 128 : (kt + 1) * 128],
                        rhs=qT[64:128, qc * S_CHUNK : (qc + 1) * S_CHUNK],
                        start=True,
                        stop=True,
                    )
                    ex = expp.tile([128, 1024], BF16, name=f"ex{pair}_{qc}_{kt}", tag="ex")
                    nc.scalar.activation(
                        ex,
                        sc,
                        mybir.ActivationFunctionType.Exp,
                        bias=mb_sb[:, kt : kt + 1],
                        scale=0.125,
                    )
                    nc.tensor.matmul(
                        av_a,
                        lhsT=v_tiles[(pair, kt)][:, 0:65],
                        rhs=ex[:, 0:512],
                        start=(kt == 0),
                        stop=(kt == N_KT - 1),
                    )
                    nc.tensor.matmul(
                        av_b,
                        lhsT=v_tiles[(pair, kt)][:, 65:130],
                        rhs=ex[:, 512:1024],
                        start=(kt == 0),
                        stop=(kt == N_KT - 1),
                    )

                # transpose back to [q, d_v], normalize, store
                stgs = [
                    stgp.tile([128, 128], F32, name=f"st{pair}_{qc}_{u}", tag="stg")
                    for u in range(4)
                ]
                for h_i, av in enumerate((av_a, av_b)):
                    avt = avtp.tile([65, S_CHUNK], F32, name=f"avt{pair}_{qc}_{h_i}", tag="avt")
                    nc.vector.tensor_copy(avt, av)
                    tp = tp_ps.tile([128, 260], F32, name=f"tp{pair}_{qc}_{h_i}", tag="tp")
                    for u in range(4):
                        nc.tensor.transpose(
                            tp[:, u * 65 : u * 65 + 65],
                            avt[:, u * 128 : (u + 1) * 128],
                            ident[0:65, 0:65],
                        )
                    for u in range(4):
                        rc = rp.tile([128, 1], F32, name=f"rc{pair}_{qc}_{h_i}_{u}", tag="rc")
                        nc.vector.reciprocal(rc, tp[:, u * 65 + 64 : u * 65 + 65])
                        nc.vector.tensor_scalar_mul(
                            stgs[u][:, h_i * 64 : (h_i + 1) * 64],
                            tp[:, u * 65 : u * 65 + 64],
                            rc,
                        )
                for u in range(4):
                    qt = qc * 4 + u
                    nc.sync.dma_start(
                        out[qt * 128 : (qt + 1) * 128, pair * 128 : (pair + 1) * 128],
                        stgs[u],
                    )


def _prep_core_inputs(pre_qs, pre_ks, pre_vs, k_mask, q_w, q_b, k_w, k_b, v_w, v_b, core):
    b = core // 2
    hh = core % 2
    cols = slice(HALF * hh, HALF * (hh + 1))

    xq = np.ascontiguousarray(pre_qs[b].T).astype(BF16_NP)
    xk = np.ascontiguousarray(pre_ks[b].T).astype(BF16_NP)
    xv = np.ascontiguousarray(pre_vs[b].T).astype(BF16_NP)
    wq = np.ascontiguousarray(q_w[:, cols]).astype(BF16_NP)
    wk = np.ascontiguousarray(k_w[:, cols]).astype(BF16_NP)

    wv_core = v_w[:, cols].astype(np.float32)
    wv = np.zeros((D_PRE, N_PAIRS * 130), dtype=np.float32)
    bv_core = v_b[cols].astype(np.float32)
    bv_ext = np.zeros(N_PAIRS * 130, dtype=np.float32)
    for p in range(N_PAIRS):
        wv[:, p * 130 : p * 130 + 64] = wv_core[:, p * 128 : p * 128 + 64]
        wv[:, p * 130 + 65 : p * 130 + 129] = wv_core[:, p * 128 + 64 : p * 128 + 128]
        bv_ext[p * 130 : p * 130 + 64] = bv_core[p * 128 : p * 128 + 64]
        bv_ext[p * 130 + 64] = 1.0
        bv_ext[p * 130 + 65 : p * 130 + 129] = bv_core[p * 128 + 64 : p * 128 + 128]
        bv_ext[p * 130 + 129] = 1.0

    bq = np.ascontiguousarray(q_b[cols].astype(np.float32).reshape(N_PAIRS, 128).T)
    bk = np.ascontiguousarray(k_b[cols].astype(np.float32).reshape(N_PAIRS, 128).T)
    bv_full = np.ascontiguousarray(np.tile(bv_ext[None, :], (128, 1)))

    # mask True -> 0.0, False -> MASK_NEG
    mbias = np.where(k_mask[b], 0.0, MASK_NEG).astype(np.float32)
    mb = np.ascontiguousarray(mbias.reshape(N_KT, 128).T)

    return {
        "xq": xq,
        "xk": xk,
        "xv": xv,
        "wq": wq,
        "wk": wk,
        "wv": wv.astype(BF16_NP),
        "bq": bq,
        "bk": bk,
        "bv": bv_full,
        "mb": mb,
    }


def kernel(pre_qs, pre_ks, pre_vs, k_mask, q_w, q_b, k_w, k_b, v_w, v_b):
    global _COMPILED
    args = (pre_qs, pre_ks, pre_vs, k_mask, q_w, q_b, k_w, k_b, v_w, v_b)
    args = tuple(np.asarray(a) for a in args)

    if _COMPILED is None:
        _COMPILED = _build_program()
    nc = _COMPILED

    in_maps = [_prep_core_inputs(*args, core=c) for c in range(N_CORES)]

    trace = bool(int(os.environ.get("BASS_KERNEL_TRACE", "0")))
    res = run_bass_kernel_spmd(
        nc,
        in_maps,
        core_ids=list(range(N_CORES)),
        trace=trace,
    )
    if trace:
        kernel.last_results = res

    out = np.empty((B, SQ, H * D_V), dtype=np.float32)
    for c in range(N_CORES):
        b = c // 2
        hh = c % 2
        out[b, :, HALF * hh : HALF * (hh + 1)] = res.results[c]["out"]
    return out


# revision 8
# speedup vs baseline: 1.1600x; 1.1600x over previous
"""Trainium2 Bass kernel for nn_AttentionSeqToMasked (dense transformer attention).

Full-input contract: kernel(**inputs) takes the unsharded numpy inputs and
returns the full [B, SQ, H*D_V] float32 output.

Sharding (8 cores): data parallel over batch (B=4 -> 2 cores per batch) x
tensor parallel over heads (16 heads -> 8 per core). Each core computes
attention for one (batch, head-half) pair; host gathers the slices.

Per-core dataflow (all matmuls bf16 inputs, fp32 PSUM accumulation):
  - Host pre-transposes activations to X^T [D_PRE, S] bf16 so the contraction
    dim (D_PRE) lands on SBUF partitions with fully-contiguous DMA loads.
  - Projections compute q^T/k^T = W^T @ X^T directly (head-dim on partitions),
    v in natural [s, d_v] layout with a ones-column appended via the weight
    matrix (W_ext column of zeros + bias 1.0).
  - Scores are computed transposed: scoresT[k, q] = kT.T @ qT, two heads
    packed into the 128x128 PE array per matmul pair (d_head=64 row groups).
  - Key-mask folds into the exp as a per-partition bias (0 or -30000);
    1/sqrt(d) folds into the exp scale. No max-subtraction is needed
    (logits are O(1) by construction; exp cannot overflow fp32).
  - AV matmul contracts exp(scores)T with [v | ones]: row 64 of the psum is
    the softmax denominator, computed for free alongside the numerator.
  - A final PE transpose returns [q, d_v+1] tiles; VectorE multiplies by the
    reciprocal denominator and the result DMAs straight to DRAM.
"""

import os

import numpy as np
import ml_dtypes

import concourse.bass as bass
import concourse.bacc as bacc
import concourse.mybir as mybir
import concourse.tile as tile
from concourse.bass_utils import run_bass_kernel_spmd
from concourse.masks import make_identity

# Problem shape (hardcoded per contract)
B, SQ, SK = 4, 2048, 2048
D_PRE = 1024
H, D_QK, D_V = 16, 64, 64
N_CORES = 8
HALF = (H // 2) * D_QK  # 512 columns of the projection handled per core
N_PAIRS = 4  # head pairs per core
S_CHUNK = 512  # moving free-dim per matmul
N_DT = D_PRE // 128  # d_pre tiles of 128
N_KT = SK // 128  # key tiles of 128
N_QC = SQ // S_CHUNK  # query chunks of 512
MASK_NEG = -30000.0

F32 = mybir.dt.float32
BF16 = mybir.dt.bfloat16
BF16_NP = np.dtype(ml_dtypes.bfloat16)

_COMPILED = None


def _build_program():
    nc = bacc.Bacc("TRN2", target_bir_lowering=False, debug=False)

    # DRAM I/O (names are the in_map keys)
    xq = nc.dram_tensor("xq", [D_PRE, SQ], BF16, kind="ExternalInput").ap()
    xk = nc.dram_tensor("xk", [D_PRE, SK], BF16, kind="ExternalInput").ap()
    xv = nc.dram_tensor("xv", [D_PRE, SK], BF16, kind="ExternalInput").ap()
    wq = nc.dram_tensor("wq", [D_PRE, HALF], BF16, kind="ExternalInput").ap()
    wk = nc.dram_tensor("wk", [D_PRE, HALF], BF16, kind="ExternalInput").ap()
    # v weights with a zero column appended per head (ones column generator)
    wv = nc.dram_tensor("wv", [D_PRE, N_PAIRS * 130], BF16, kind="ExternalInput").ap()
    bq = nc.dram_tensor("bq", [128, N_PAIRS], F32, kind="ExternalInput").ap()
    bk = nc.dram_tensor("bk", [128, N_PAIRS], F32, kind="ExternalInput").ap()
    bv = nc.dram_tensor("bv", [128, N_PAIRS * 130], F32, kind="ExternalInput").ap()
    mb = nc.dram_tensor("mb", [128, N_KT], F32, kind="ExternalInput").ap()
    out = nc.dram_tensor("out", [SQ, HALF], F32, kind="ExternalOutput").ap()

    with tile.TileContext(nc) as tc:
        _emit(tc, xq, xk, xv, wq, wk, wv, bq, bk, bv, mb, out)

    nc.compile()
    return nc


def _emit(tc, xq, xk, xv, wq, wk, wv, bq, bk, bv, mb, out):
    nc = tc.nc
    from contextlib import ExitStack

    with ExitStack() as ctx:
        # ---- pools ----
        xp = ctx.enter_context(tc.tile_pool(name="x", bufs=3 * N_DT))
        wp = ctx.enter_context(tc.tile_pool(name="w", bufs=1))
        cp = ctx.enter_context(tc.tile_pool(name="const", bufs=1))
        qkvp = ctx.enter_context(tc.tile_pool(name="qkv", bufs=1))
        expp = ctx.enter_context(tc.tile_pool(name="exp", bufs=3))
        avtp = ctx.enter_context(tc.tile_pool(name="avt", bufs=2))
        stgp = ctx.enter_context(tc.tile_pool(name="stg", bufs=4))
        rp = ctx.enter_context(tc.tile_pool(name="recip", bufs=4))

        proj_ps = ctx.enter_context(tc.tile_pool(name="proj_ps", bufs=1, space="PSUM"))
        sc_ps = ctx.enter_context(tc.tile_pool(name="sc_ps", bufs=2, space="PSUM"))
        av_ps = ctx.enter_context(tc.tile_pool(name="av_ps", bufs=2, space="PSUM"))
        tp_ps = ctx.enter_context(tc.tile_pool(name="tp_ps", bufs=1, space="PSUM"))

        # ---- constants ----
        ident = cp.tile([128, 128], F32, name="ident")
        make_identity(nc, ident)
        mb_sb = cp.tile([128, N_KT], F32, name="mb_sb")
        nc.sync.dma_start(mb_sb, mb)
        bq_sb = cp.tile([128, N_PAIRS], F32, name="bq_sb")
        nc.sync.dma_start(bq_sb, bq)
        bk_sb = cp.tile([128, N_PAIRS], F32, name="bk_sb")
        nc.sync.dma_start(bk_sb, bk)
        bv_sb = cp.tile([128, N_PAIRS * 130], F32, name="bv_sb")
        nc.sync.dma_start(bv_sb, bv)

        # ---- streamed loads, ordered so the q projection can start after
        # only xq+wq have landed (startup latency) ----
        def load_x(xap, pfx):
            ts = []
            for dt_i in range(N_DT):
                t = xp.tile([128, SQ], BF16, name=f"{pfx}{dt_i}", tag="x")
                nc.sync.dma_start(t, xap[dt_i * 128 : (dt_i + 1) * 128, :])
                ts.append(t)
            return ts

        def load_w(wap, pfx, width):
            ts = []
            for dt_i in range(N_DT):
                t = wp.tile([128, width], BF16, name=f"{pfx}{dt_i}", tag=f"{pfx}{dt_i}")
                nc.sync.dma_start(t, wap[dt_i * 128 : (dt_i + 1) * 128, :])
                ts.append(t)
            return ts

        wq_sb = load_w(wq, "wq", HALF)
        xq_sb = load_x(xq, "xq")
        wk_sb = load_w(wk, "wk", HALF)
        xk_sb = load_x(xk, "xk")
        wv_sb = load_w(wv, "wv", N_PAIRS * 130)
        xv_sb = load_x(xv, "xv")

        v_tiles = {}  # (pair, kt) -> [128, 130] bf16 tile

        def emit_v_proj(pair, sts):
            # v (natural layout) + ones column for key tiles `sts` of `pair`
            for st in sts:
                ps = proj_ps.tile([128, 512], F32, name=f"vps{pair}_{st}", tag="proj")
                for dt_i in range(N_DT):
                    nc.tensor.matmul(
                        ps[:, 0:130],
                        lhsT=xv_sb[dt_i][:, st * 128 : (st + 1) * 128],
                        rhs=wv_sb[dt_i][:, pair * 130 : (pair + 1) * 130],
                        start=(dt_i == 0),
                        stop=(dt_i == N_DT - 1),
                    )
                vt = qkvp.tile([128, 130], BF16, name=f"v{pair}_{st}", tag="v", bufs=4 * N_KT)
                nc.vector.tensor_add(
                    vt,
                    ps[:, 0:130],
                    bv_sb[:, pair * 130 : (pair + 1) * 130],
                )
                v_tiles[(pair, st)] = vt

        qkT = {}  # (pfx, pair) -> [128, SQ] bf16 tile

        def emit_qk_proj(pair, pfx, scs):
            # out^T[hd128, s] accumulated over d tiles; bias added on the copy
            if (pfx, pair) not in qkT:
                qkT[(pfx, pair)] = qkvp.tile(
                    [128, SQ], BF16, name=f"{pfx}T{pair}", tag=f"{pfx}T", bufs=3
                )
            dst = qkT[(pfx, pair)]
            w_sb = wq_sb if pfx == "q" else wk_sb
            b_sb = bq_sb if pfx == "q" else bk_sb
            x_sb = xq_sb if pfx == "q" else xk_sb
            for qc in scs:
                ps = proj_ps.tile([128, S_CHUNK], F32, name=f"{pfx}ps{pair}_{qc}", tag="proj")
                for dt_i in range(N_DT):
                    nc.tensor.matmul(
                        ps,
                        lhsT=w_sb[dt_i][:, pair * 128 : (pair + 1) * 128],
                        rhs=x_sb[dt_i][:, qc * S_CHUNK : (qc + 1) * S_CHUNK],
                        start=(dt_i == 0),
                        stop=(dt_i == N_DT - 1),
                    )
                nc.vector.tensor_scalar_add(
                    dst[:, qc * S_CHUNK : (qc + 1) * S_CHUNK],
                    ps,
                    b_sb[:, pair : pair + 1],
                )

        def emit_proj_slice(pair, i):
            # 1/4 of a pair's projection work: interleaved into the previous
            # pair's attention loop as TensorE filler
            if i == 0:
                emit_qk_proj(pair, "q", range(N_QC))
            elif i == 1:
                emit_qk_proj(pair, "k", range(N_QC))
            elif i == 2:
                emit_v_proj(pair, range(0, N_KT // 2))
            else:
                emit_v_proj(pair, range(N_KT // 2, N_KT))

        # prologue: full projection for pair 0
        for i in range(4):
            emit_proj_slice(0, i)

        for pair in range(N_PAIRS):
            qT = qkT[("q", pair)]
            kT = qkT[("k", pair)]

            # ---- attention for this pair ----
            for qc in range(N_QC):
                av_a = av_ps.tile([65, S_CHUNK], F32, name=f"ava{pair}_{qc}", tag="av")
                av_b = av_ps.tile([65, S_CHUNK], F32, name=f"avb{pair}_{qc}", tag="av")
                for kt in range(N_KT):
                    sc = sc_ps.tile([128, 1024], F32, name=f"sc{pair}_{qc}_{kt}", tag="sc")
                    # scoresT for heads A and B, packed in PE row groups
                    nc.tensor.matmul(
                        sc[:, 0:512],
                        lhsT=kT[0:64, kt * 128 : (kt + 1) * 128],
                        rhs=qT[0:64, qc * S_CHUNK : (qc + 1) * S_CHUNK],
                        start=True,
                        stop=True,
                    )
                    nc.tensor.matmul(
                        sc[:, 512:1024],
                        lhsT=kT[64:128, kt * 128 : (kt + 1) * 128],
                        rhs=qT[64:128, qc * S_CHUNK : (qc + 1) * S_CHUNK],
                        start=True,
                        stop=True,
                    )
                    ex = expp.tile([128, 1024], BF16, name=f"ex{pair}_{qc}_{kt}", tag="ex")
                    nc.scalar.activation(
                        ex,
                        sc,
                        mybir.ActivationFunctionType.Exp,
                        bias=mb_sb[:, kt : kt + 1],
                        scale=0.125,
                    )
                    nc.tensor.matmul(
                        av_a,
                        lhsT=v_tiles[(pair, kt)][:, 0:65],
                        rhs=ex[:, 0:512],
                        start=(kt == 0),
                        stop=(kt == N_KT - 1),
                    )
                    nc.tensor.matmul(
                        av_b,
                        lhsT=v_tiles[(pair, kt)][:, 65:130],
                        rhs=ex[:, 512:1024],
                        start=(kt == 0),
                        stop=(kt == N_KT - 1),
                    )

                # transpose back to [q, d_v], normalize, store
                stgs = [
                    stgp.tile([128, 128], F32, name=f"st{pair}_{qc}_{u}", tag="stg")
                    for u in range(4)
                ]
                for h_i, av in enumerate((av_a, av_b)):
                    avt = avtp.tile([65, S_CHUNK], F32, name=f"avt{pair}_{qc}_{h_i}", tag="avt")
                    nc.vector.tensor_copy(avt, av)
                    tp = tp_ps.tile([128, 260], F32, name=f"tp{pair}_{qc}_{h_i}", tag="tp")
                    for u in range(4):
                        nc.tensor.transpose(
                            tp[:, u * 65 : u * 65 + 65],
                            avt[:, u * 128 : (u + 1) * 128],
                            ident[0:65, 0:65],
                        )
                    for u in range(4):
                        rc = rp.tile([128, 1], F32, name=f"rc{pair}_{qc}_{h_i}_{u}", tag="rc")
                        nc.vector.reciprocal(rc, tp[:, u * 65 + 64 : u * 65 + 65])
                        nc.vector.tensor_scalar_mul(
                            stgs[u][:, h_i * 64 : (h_i + 1) * 64],
                            tp[:, u * 65 : u * 65 + 64],
                            rc,
                        )
                for u in range(4):
                    qt = qc * 4 + u
                    nc.sync.dma_start(
                        out[qt * 128 : (qt + 1) * 128, pair * 128 : (pair + 1) * 128],
                        stgs[u],
                    )

                # TensorE filler: next pair's projections, one slice per qc
                if pair + 1 < N_PAIRS:
                    emit_proj_slice(pair + 1, qc)


def _prep_core_inputs(pre_qs, pre_ks, pre_vs, k_mask, q_w, q_b, k_w, k_b, v_w, v_b, core):
    b = core // 2
    hh = core % 2
    cols = slice(HALF * hh, HALF * (hh + 1))

    xq = np.ascontiguousarray(pre_qs[b].T).astype(BF16_NP)
    xk = np.ascontiguousarray(pre_ks[b].T).astype(BF16_NP)
    xv = np.ascontiguousarray(pre_vs[b].T).astype(BF16_NP)
    wq = np.ascontiguousarray(q_w[:, cols]).astype(BF16_NP)
    wk = np.ascontiguousarray(k_w[:, cols]).astype(BF16_NP)

    wv_core = v_w[:, cols].astype(np.float32)
    wv = np.zeros((D_PRE, N_PAIRS * 130), dtype=np.float32)
    bv_core = v_b[cols].astype(np.float32)
    bv_ext = np.zeros(N_PAIRS * 130, dtype=np.float32)
    for p in range(N_PAIRS):
        wv[:, p * 130 : p * 130 + 64] = wv_core[:, p * 128 : p * 128 + 64]
        wv[:, p * 130 + 65 : p * 130 + 129] = wv_core[:, p * 128 + 64 : p * 128 + 128]
        bv_ext[p * 130 : p * 130 + 64] = bv_core[p * 128 : p * 128 + 64]
        bv_ext[p * 130 + 64] = 1.0
        bv_ext[p * 130 + 65 : p * 130 + 129] = bv_core[p * 128 + 64 : p * 128 + 128]
        bv_ext[p * 130 + 129] = 1.0

    bq = np.ascontiguousarray(q_b[cols].astype(np.float32).reshape(N_PAIRS, 128).T)
    bk = np.ascontiguousarray(k_b[cols].astype(np.float32).reshape(N_PAIRS, 128).T)
    bv_full = np.ascontiguousarray(np.tile(bv_ext[None, :], (128, 1)))

    # mask True -> 0.0, False -> MASK_NEG
    mbias = np.where(k_mask[b], 0.0, MASK_NEG).astype(np.float32)
    mb = np.ascontiguousarray(mbias.reshape(N_KT, 128).T)

    return {
        "xq": xq,
        "xk": xk,
        "xv": xv,
        "wq": wq,
        "wk": wk,
        "wv": wv.astype(BF16_NP),
        "bq": bq,
        "bk": bk,
        "bv": bv_full,
        "mb": mb,
    }


def kernel(pre_qs, pre_ks, pre_vs, k_mask, q_w, q_b, k_w, k_b, v_w, v_b):
    global _COMPILED
    args = (pre_qs, pre_ks, pre_vs, k_mask, q_w, q_b, k_w, k_b, v_w, v_b)
    args = tuple(np.asarray(a) for a in args)

    if _COMPILED is None:
        _COMPILED = _build_program()
    nc = _COMPILED

    in_maps = [_prep_core_inputs(*args, core=c) for c in range(N_CORES)]

    trace = bool(int(os.environ.get("BASS_KERNEL_TRACE", "0")))
    res = run_bass_kernel_spmd(
        nc,
        in_maps,
        core_ids=list(range(N_CORES)),
        trace=trace,
    )
    if trace:
        kernel.last_results = res

    out = np.empty((B, SQ, H * D_V), dtype=np.float32)
    for c in range(N_CORES):
        b = c // 2
        hh = c % 2
        out[b, :, HALF * hh : HALF * (hh + 1)] = res.results[c]["out"]
    return out
